# revision 23
# baseline (speedup 1.0000x reference)
"""Multi-head attention (B=1, S=4096, H=16, D=64) on 8 Trainium2 NeuronCores.

Sharding: 2 heads per core (pure head-parallel, no cross-core comms).

v2 design notes (vs the v1 PE-transpose pipeline):

- Host-side marshaling: Q/K are pre-transposed to [W=128, S] and pre-cast to
  bf16 on the host; V is pre-arranged into the PV-weights layout
  [128 kk, n_sk, 2 heads, 80] f16 (64 dims + ones column + 15 zero pad) so the
  device does no prep transposes, casts, or memsets at all.

- Every matmul runs in the same (64,128) row-tiled PE mode.  The v1 kernel
  alternated (64,128) QK pairs with (128,128) PV/transpose matmuls; each mode
  switch drains the PE array (~110ns, 2x/step ~= 56us total).  Here PV is
  split into two half-contraction row tiles (T0: kk 0-63, T8: kk 64-127) that
  stream concurrently pairwise, so the whole kernel stays in one mode.
  Per-step PE emission order [A-T0, A-T8, B-T8, QK-T0, QK-T8, B-T0] staggers
  same-PSUM-bank writers (each bank's two half-writers are >=1 matmul apart
  on the later writer's tile) per the row-tiling PSUM rule.

- oT is [80, 512] (64 dims + denominator + 15 zeros from the VP pad), so the
  drain can XBAR-DMA-transpose [80,128] f16 slices (p%16==0) with no PE
  transposes and no uninitialized reads.  Normalize on DVE from SBUF.

- exp dispatch per chunk: ScalarE table Exp (exact), or DVE fp16 paths:
  'corr' = 3-pass Schraudolph + quadratic mantissa fix via the custom DVE op
  (0.36% max err), 'schr' = 1-pass Schraudolph (3.9% max, zero-mean; cheap).
  Probs are fp16 (bits written by tensor_scalar/custom-op, bitcast to f16).
"""

import sys

for _p in ("/opt/trn_rl_repo", "/root/.axon_site/_ro/trn_rl_repo"):
    if _p not in sys.path:
        sys.path.append(_p)

import numpy as np

_B, _S, _H, _D = 1, 4096, 16, 64
_NCORES = 8
_HPC = _H // _NCORES  # heads per core

_LOG2E = float(np.log2(np.e))
_SCALE = 0.125               # 1/sqrt(D)
_EXPB = -float(np.log(16.0))  # probs stored as exp(s)/16 (fp16 headroom)

# fp16 Schraudolph constants (J = round(s*CA + CB) bitcast f16 ~ exp(s/8)/16)
_CA = 1024.0 * _LOG2E / 8.0           # 184.6649652337873
_CB_CORR = 11263.815588854253         # corrected path: C ~= 0 (+zero-mean tilt)
_CB_SCHR = 11205.074                  # 1-pass path: zero-mean centering
_KI = -0.5 + 1.0 / 2048.0             # I = floor(J/1024) via round-to-nearest
_GAMMA = 0.00033203125                # 0.34/1024 quadratic mantissa fix


def register_pass2_op():
    import concourse.dve_ops as dve_ops
    from concourse.dve_ops import DveOp
    from concourse.dve_spec import Spec, Src0, Src1, C0, C1, C2, AluOp, Bin, lower
    from concourse.dve_uop import DveOpSpec

    name = "EXPFIX_ANT"
    if name in dve_ops._SUB_OPCODE_FOR_NAME:
        return next(op for op in dve_ops.OPS if op.name == name)
    mult, add, sub = AluOp.MULTIPLY, AluOp.ADD, AluOp.SUBTRACT
    t = Bin(mult, Src0, C0)      # 1024*I
    u = Bin(sub, Src1, t)        # J - 1024*I
    w = Bin(add, u, C1)          # = w (C1=0)
    g = Bin(sub, C0, w)          # 1024 - w
    h = Bin(mult, g, w)
    G = Bin(mult, h, C2)         # * -gamma
    bits = Bin(add, Src1, G)

    def ref(in0, in1, c0, c1, c2):
        a0 = np.asarray(in0, np.float32)
        a1 = np.asarray(in1, np.float32)
        w = (a1 - a0 * c0) + c1
        return (a1 + (c0 - w) * w * c2).astype(np.float32)

    spec = Spec(body=bits, reference=ref)
    row = max(dve_ops._SUB_OPCODE_FOR_NAME.values()) + 1
    dve_ops._SUB_OPCODE_FOR_NAME[name] = row
    uops = lower(spec, ver="v3")
    sha = DveOpSpec(name=name, opcode=row, uops=uops, rd1_en=True).sha("v3")
    op = DveOp(name, spec, subdim=False, uops_sha={"v3": sha}, perf_en={"v3": True})
    dve_ops.OPS.append(op)
    dve_ops.CUSTOM_DVE_SPECS[name] = spec
    return op


def build_program(S=_S, n_heads=_HPC, blk=512, corr_cs=None, corr_cs_b0=None,
                  schr_cs=None, schr_cs_b0=None, hw_round=True):
    """Single-core Bass program (SPMD), uniform (64,128)-tiled PE mode.

    hw_round: hardware float->int converts round-to-nearest; CoreSim
    truncates.  The DVE exp constants depend on which one applies."""
    import concourse.tile as tile
    from concourse import bacc, mybir

    rfix = 0.0 if hw_round else 0.5
    cb_corr = _CB_CORR + rfix
    cb_schr = _CB_SCHR + rfix
    kI = _KI if hw_round else (1.0 / 2048.0)

    f32 = mybir.dt.float32
    bf16 = mybir.dt.bfloat16
    f16 = mybir.dt.float16
    i16 = mybir.dt.int16
    u16 = mybir.dt.uint16
    D = _D
    W = n_heads * D
    n_sk = S // 128
    n_blk = S // blk
    n_j = blk // 128
    assert n_heads == 2 and W == 128 and blk == 512

    exp_op = register_pass2_op()

    if corr_cs is None:
        corr_cs = (1, 5, 13, 17, 25, 29) if S == _S else ()
    if corr_cs_b0 is None:
        corr_cs_b0 = (17, 25) if S == _S else ()
    if schr_cs is None:
        schr_cs = (7, 11, 19, 23) if S == _S else ()
    if schr_cs_b0 is None:
        schr_cs_b0 = (21, 29) if S == _S else ()

    nc = bacc.Bacc("TRN2", target_bir_lowering=False, debug=False)
    qt_in = nc.dram_tensor("qt", [W, S], bf16, kind="ExternalInput")
    kt_in = nc.dram_tensor("kt", [W, S], bf16, kind="ExternalInput")
    vp_in = nc.dram_tensor("vp", [128, n_sk * n_heads * 80], f16,
                           kind="ExternalInput")
    out = nc.dram_tensor("out", [S, W], f32, kind="ExternalOutput")

    with tile.TileContext(nc) as tc:
        with (
            tc.tile_pool(name="singles", bufs=1) as singles,
            tc.tile_pool(name="qkt", bufs=1) as qkt,
            tc.tile_pool(name="expool", bufs=10) as expool,
            tc.tile_pool(name="ijpool", bufs=2) as ijpool,
            tc.tile_pool(name="osb", bufs=3) as osb,
            tc.tile_pool(name="tpd", bufs=5) as tpd,
            tc.tile_pool(name="outb", bufs=5) as outb,
            tc.tile_pool(name="small", bufs=4) as small,
            tc.tile_pool(name="ps_a", bufs=2, space="PSUM") as ps_act,
            tc.tile_pool(name="ps_o", bufs=1, space="PSUM") as ps_out,
        ):
            exp_bias = singles.tile([128, 1], f32, tag="expb")
            nc.vector.memset(exp_bias, _EXPB)

            # preload the Exp table set early (2.7us, off critical path)
            dum = small.tile([128, 1], f32, tag="rec", name="dum")
            nc.vector.memset(dum, 0.0)
            dum2 = small.tile([128, 1], f32, tag="rec", name="dum2")
            nc.scalar.activation(dum2, dum, mybir.ActivationFunctionType.Exp)

            # warmup weights (zeros) for dummy 64x128-mode matmul pairs
            wt = singles.tile([128, 128], bf16, tag="wt")
            nc.vector.memset(wt, 0.0)

            # ---- SBUF-resident operands ----
            QT = qkt.tile([W, S], bf16, tag="qt")
            KT = qkt.tile([W, S], bf16, tag="kt")
            VP = qkt.tile([128, n_sk, n_heads, 80], f16, tag="vp")
            vp_view = vp_in[:, :].rearrange("p (c h x) -> p c h x", h=n_heads, x=80)

            # ---- input DMA schedule ----
            # first QK needs kt cols 0:256 + qt block 0; stream the rest.
            if S == _S:
                nc.sync.dma_start(out=KT[:, 0:256], in_=kt_in[:, 0:256])
                nc.gpsimd.dma_start(out=QT[:, 0:blk], in_=qt_in[:, 0:blk])
                nc.sync.dma_start(out=KT[:, 256:1024], in_=kt_in[:, 256:1024])
                nc.gpsimd.dma_start(out=VP[:, 0:8], in_=vp_view[:, 0:8])
                nc.sync.dma_start(out=KT[:, 1024:2048], in_=kt_in[:, 1024:2048])
                nc.gpsimd.dma_start(out=VP[:, 8:16], in_=vp_view[:, 8:16])
                pend_loads = [
                    (1, lambda: nc.gpsimd.dma_start(
                        out=KT[:, 2048:3072], in_=kt_in[:, 2048:3072])),
                    (2, lambda: nc.gpsimd.dma_start(
                        out=VP[:, 16:24], in_=vp_view[:, 16:24])),
                    (4, lambda: nc.gpsimd.dma_start(
                        out=KT[:, 3072:4096], in_=kt_in[:, 3072:4096])),
                    (6, lambda: nc.gpsimd.dma_start(
                        out=VP[:, 24:32], in_=vp_view[:, 24:32])),
                ]
                for b in range(1, n_blk):
                    pend_loads.append(
                        (32 * b - 28, lambda b=b: nc.gpsimd.dma_start(
                            out=QT[:, b * blk:(b + 1) * blk],
                            in_=qt_in[:, b * blk:(b + 1) * blk]))
                    )
            else:
                nc.sync.dma_start(out=KT, in_=kt_in[:, :])
                nc.scalar.dma_start(out=QT, in_=qt_in[:, :])
                nc.sync.dma_start(out=VP, in_=vp_view)
                pend_loads = []

            # ---- warmup: dummy (64,128)-tiled pairs to set mode + wake HAM ----
            warm = ps_act.tile([128, 2 * blk], f32, tag="psa", name="warm")
            for i in range(4):
                sl = slice(i * 128, i * 128 + 128)
                sl2 = slice(blk + i * 128, blk + i * 128 + 128)
                nc.tensor.matmul(warm[:, sl], lhsT=wt[0:64, :],
                                 rhs=wt[0:64, 0:128], start=True, stop=True)
                nc.tensor.matmul(warm[:, sl2], lhsT=wt[64:128, :],
                                 rhs=wt[64:128, 0:128], start=True, stop=True)

            # ---- main pipeline ----
            steps = [(b, c) for b in range(n_blk) for c in range(n_sk)]
            ps_tiles = {}

            def kind_of(idx):
                b, c = steps[idx]
                if c in (corr_cs_b0 if b == 0 else corr_cs):
                    return "corr"
                if c in (schr_cs_b0 if b == 0 else schr_cs):
                    return "schr"
                return "act"

            def emit_qk(idx):
                b, c = steps[idx]
                t = ps_act.tile([128, 2 * blk], f32, tag="psa",
                                name=f"psa_{b}_{c}")
                ps_tiles[(b, c)] = t
                # T0 (rows 0-63): head 0; T8 (rows 64-127): head 1 — concurrent
                nc.tensor.matmul(
                    t[:, 0:blk],
                    lhsT=KT[0:64, c * 128:(c + 1) * 128],
                    rhs=QT[0:64, b * blk:(b + 1) * blk],
                    start=True, stop=True,
                )
                nc.tensor.matmul(
                    t[:, blk:2 * blk],
                    lhsT=KT[64:128, c * 128:(c + 1) * 128],
                    rhs=QT[64:128, b * blk:(b + 1) * blk],
                    start=True, stop=True,
                )

            # ---- drain machinery ----
            drain_q = []
            osb_t = {}
            tpd_t = {}
            rec_t = {}
            obm_t = {}

            def queue_drain(now, b, h, oT_lo, oT_hi):
                # Free the lo bank NOW (next block's A-pass reuses it in ~3
                # steps); the rest is step-stamped so nothing on Vector ever
                # blocks on the ~5us of serialized XBAR transposes.
                key = (b, h)
                lo_sb = osb.tile([80, blk], f32, tag="losb",
                                 name=f"losb_{h}_{b}")
                nc.vector.tensor_copy(lo_sb, oT_lo)
                osb_t[key] = (lo_sb, oT_hi)
                dly = 2 * (h > 0)
                stamps = (1, 2, 10, 12) if b < n_blk - 1 else (1, 2, 3, 4)
                for piece, at in enumerate(stamps):
                    drain_q.append((now + at + dly, b, h, piece))
                drain_q.sort(key=lambda e: e[0])

            def emit_drain_pieces(now):
                while drain_q and drain_q[0][0] <= now:
                    _, b, h, piece = drain_q.pop(0)
                    key = (b, h)
                    if piece == 0:
                        # frees the hi bank; PSUM has one DVE read port so
                        # the lo half comes from its SBUF staging copy.
                        lo_sb, oT_hi = osb_t[key]
                        o_sb = osb.tile([80, blk], f16, tag="osb",
                                        name=f"osb_{h}_{b}")
                        nc.vector.tensor_tensor(
                            out=o_sb, in0=oT_hi, in1=lo_sb,
                            op=mybir.AluOpType.add)
                        osb_t[key] = o_sb
                    elif piece == 1:
                        tpd_t[key] = tpd.tile([128, n_j, 80], f16, tag="tpd",
                                              name=f"tpd_{h}_{b}")
                        obm_t[key] = outb.tile([128, n_j, 64], f32, tag="obm",
                                               name=f"obm_{h}_{b}")
                        xeng = nc.sync if b < n_blk - 1 else (
                            nc.sync if h == 0 else nc.scalar)
                        for j in range(n_j):
                            xeng.dma_start(
                                out=tpd_t[key][:, j, :],
                                in_=osb_t[key][0:80, j * 128:(j + 1) * 128],
                                transpose=True)
                    elif piece == 2:
                        rec_t[key] = small.tile([128, n_j, 1], f32, tag="rec",
                                                name=f"rec_{h}_{b}")
                        nc.vector.reciprocal(rec_t[key],
                                             tpd_t[key][:, :, 64:65])
                    else:
                        for j in range(n_j):
                            nc.vector.tensor_scalar_mul(
                                obm_t[key][:, j, :], tpd_t[key][:, j, 0:64],
                                rec_t[key][:, j, :])
                        P0 = h * 64
                        nc.gpsimd.dma_start(
                            out=out[b * blk:(b + 1) * blk,
                                    P0:P0 + 64].rearrange(
                                "(j p) d -> p j d", p=128),
                            in_=obm_t[key],
                        )

            # ---- exp dispatch ----
            ex_of = {}     # (b, c) -> f16 AP [128, 2*blk] (kk x [h0 | h1])
            pend_p2 = []

            def emit_exp(idx):
                b, c = steps[idx]
                t = ps_tiles.pop((b, c))
                k = kind_of(idx)
                if k == "act":
                    ex = expool.tile([128, 2 * blk], f16, tag="ex",
                                     name=f"ex_{idx}")
                    nc.scalar.activation(
                        ex, t, mybir.ActivationFunctionType.Exp,
                        scale=_SCALE, bias=exp_bias,
                    )
                    ex_of[(b, c)] = ex
                    return
                if k == "schr":
                    Jt = expool.tile([128, 2 * blk], u16, tag="ex",
                                     name=f"exJ_{idx}")
                    nc.vector.tensor_scalar(
                        out=Jt, in0=t, scalar1=_CA, scalar2=cb_schr,
                        op0=mybir.AluOpType.mult, op1=mybir.AluOpType.add)
                    ex_of[(b, c)] = Jt[:, :].bitcast(f16)
                    return
                Jt = ijpool.tile([128, 2 * blk], u16, tag="J", name=f"J_{idx}")
                nc.vector.tensor_scalar(
                    out=Jt, in0=t, scalar1=_CA, scalar2=cb_corr,
                    op0=mybir.AluOpType.mult, op1=mybir.AluOpType.add)
                It = ijpool.tile([128, 2 * blk], i16, tag="I", name=f"I_{idx}")
                nc.vector.tensor_scalar(
                    out=It, in0=Jt, scalar1=1.0 / 1024.0, scalar2=kI,
                    op0=mybir.AluOpType.mult, op1=mybir.AluOpType.add)
                Pt = expool.tile([128, 2 * blk], u16, tag="ex", name=f"P_{idx}")

                def emit_p2(It=It, Jt=Jt, Pt=Pt, b=b, c=c):
                    nc.vector._custom_dve(
                        exp_op, out=Pt, in0=It, in1=Jt,
                        s0=1024.0, s1=0.0, imm2=-_GAMMA)
                    ex_of[(b, c)] = Pt[:, :].bitcast(f16)

                pend_p2.append((idx + 2, emit_p2))

            def flush_p2(now):
                while pend_p2 and pend_p2[0][0] <= now:
                    pend_p2.pop(0)[1]()

            # ---- PV: row-tiled half-contraction passes ----
            # Each (h, half) accumulates into its OWN psum bank: cross-tile
            # accumulation into one bank fails on hw, and one-writer-per-bank
            # also removes every same-bank concurrency hazard.
            pend_pv = []
            oT_t = {}
            pv_count = {}

            def get_oT(b, h, half):
                key = (b, h, half)
                if key not in oT_t:
                    oT_t[key] = ps_out.tile(
                        [80, blk], f32, tag=f"oT{h}{half}",
                        name=f"oT_{h}_{half}_{b}", bufs=1
                    )
                    pv_count[key] = 0
                return oT_t[key]

            def pv_half(b, c, h, half):
                """One [64kk x 80] x [64kk x 512] matmul; half 0 = T0, 1 = T8."""
                oT = get_oT(b, h, half)
                ex = ex_of[(b, c)]
                p = half * 64
                key = (b, h, half)
                pv_count[key] += 1
                cnt = pv_count[key]
                nc.tensor.matmul(
                    oT,
                    lhsT=VP[p:p + 64, c, h, :],
                    rhs=ex[p:p + 64, h * blk:(h + 1) * blk],
                    start=(cnt == 1),
                    stop=(cnt == n_sk),
                )
                if (pv_count.get((b, h, 0)) == n_sk
                        and pv_count.get((b, h, 1)) == n_sk):
                    del pv_count[(b, h, 0)], pv_count[(b, h, 1)]
                    queue_drain(cur_step[0], b, h,
                                oT_t.pop((b, h, 0)), oT_t.pop((b, h, 1)))

            def pop_ready_pv(now):
                if pend_pv and pend_pv[0][0] <= now:
                    return pend_pv.pop(0)[1:]
                return None

            # ---- main loop ----
            # PE per-step order: A-T0, A-T8, B-T8, B-T0, QK(idx+1).
            # (A = [T0:h0-low, T8:h1-high], B = [T8:h0-high, T0:h1-low]);
            # adjacent instructions on opposite tiles stream concurrently.
            emit_qk(0)
            exp_done = set()
            cur_step = [0]

            def emit_exp_once(i):
                if i not in exp_done:
                    exp_done.add(i)
                    emit_exp(i)

            for idx, (b, c) in enumerate(steps):
                cur_step[0] = idx
                while pend_loads and pend_loads[0][0] <= idx:
                    pend_loads.pop(0)[1]()
                emit_exp_once(idx)
                flush_p2(idx)
                # QK first: it sits on the exp->slot critical cycle; the PV
                # pairs fill the remaining PE time outside that cycle.
                if idx + 1 < len(steps):
                    emit_qk(idx + 1)
                    if kind_of(idx + 1) == "corr":
                        emit_exp_once(idx + 1)
                emit_drain_pieces(idx)
                pv = pop_ready_pv(idx)
                if pv is not None:
                    # pair = two kk-halves of ONE head: identical moving
                    # free addresses (SBUF reads are lockstep across
                    # partitions), so the T0/T8 tiles stream concurrently.
                    pb, pc = pv
                    pv_half(pb, pc, 0, 0)   # h0 kk-low  (T0)
                    pv_half(pb, pc, 0, 1)   # h0 kk-high (T8, concurrent)
                    pv_half(pb, pc, 1, 0)   # h1 kk-low  (T0)
                    pv_half(pb, pc, 1, 1)   # h1 kk-high (T8, concurrent)
                    del ex_of[(pb, pc)]
                lag = 5 if kind_of(idx) == "corr" else 3
                if b == n_blk - 1 and c >= n_sk - 6:
                    lag = 2
                pend_pv.append((idx + lag, b, c))

            tail = len(steps)
            while pend_p2 or pend_pv:
                cur_step[0] = tail
                flush_p2(tail)
                pv = pop_ready_pv(tail)
                if pv is not None:
                    pb, pc = pv
                    pv_half(pb, pc, 0, 0)
                    pv_half(pb, pc, 0, 1)
                    pv_half(pb, pc, 1, 0)
                    pv_half(pb, pc, 1, 1)
                    del ex_of[(pb, pc)]
                emit_drain_pieces(tail)
                tail += 1
            while drain_q:
                emit_drain_pieces(tail)
                tail += 1
    nc.finalize()
    return nc


def host_inputs(query, key, value, S=_S, n_heads=_HPC):
    """FULL f32 inputs -> per-core pre-transposed/cast input maps."""
    import ml_dtypes

    w = n_heads * _D
    n_sk = S // 128
    in_maps = []
    for cidx in range(_NCORES):
        sl = slice(cidx * w, (cidx + 1) * w)
        q = query[0][:, sl]     # [S, 128]
        k = key[0][:, sl]
        v = value[0][:, sl]
        qT = np.ascontiguousarray(q.T).astype(ml_dtypes.bfloat16)
        kT = np.ascontiguousarray(k.T).astype(ml_dtypes.bfloat16)
        vp = np.zeros((128, n_sk, n_heads, 80), np.float32)
        vr = v.reshape(n_sk, 128, n_heads, _D).transpose(1, 0, 2, 3)
        vp[:, :, :, 0:_D] = vr
        vp[:, :, :, _D] = 1.0
        in_maps.append({
            "qt": qT,
            "kt": kT,
            "vp": vp.reshape(128, n_sk * n_heads * 80).astype(np.float16),
        })
    return in_maps


def kernel(query, key, value, trace=False, tmpdir=None, **build_kwargs):
    from concourse.bass_utils import run_bass_kernel_spmd

    query = np.asarray(query, dtype=np.float32)
    key = np.asarray(key, dtype=np.float32)
    value = np.asarray(value, dtype=np.float32)

    nc = build_program(**build_kwargs)
    in_maps = host_inputs(query, key, value)
    res = run_bass_kernel_spmd(
        nc, in_maps, list(range(_NCORES)), trace=trace, tmpdir=tmpdir
    )
    full = np.concatenate([res.results[c]["out"] for c in range(_NCORES)], axis=1)
    out = full[None].astype(np.float32)
    if trace:
        return out, res
    return out
